# revision 46
# baseline (speedup 1.0000x reference)
"""Trainium2 Bass kernel for nn_AttentionLayer (B=16, V=1024, D=512, H=8, MAXHOP=8).

Sharding: data-parallel over batch B across 8 NeuronCores (2 batches/core).
The relative-position bias is applied in EXP SPACE: P = exp(S) * expB where
expB = exp(rpe)[hop].  Core c builds head c's expB table on-chip (9-pass
select-accumulate on the Vector engine), then two AllGather halves
distribute all 8 heads to every core (optionally in fp8 to halve the
collective volume - the table has only 9 distinct values per head, so fp8
costs one ~3% quantization of those values, no accumulation error).
Because the bias is multiplicative-after-exp, the S matmuls and the exp()
do NOT depend on the collective - only the elementwise multiply does, so
the PE/ScalarE stream runs ahead of the gather.

Per-core math (transposed-score layout, softmax on native axes):
  qT/kT = (W @ x^T) per head      [HD, tokens]  (bf16, fp32 accum; host
                                   pre-casts x and weights to bf16)
  S_T[j,i] = k_j . q_i * scale    (2x N=512 matmuls per head-tile)
  E_T = exp(S_T)                  (ScalarE, PSUM -> SBUF bf16, one op/tile)
  P_T = E_T * expB_T              (VectorE)
  att_T[d,i] (+denom row) = [v|1]^T @ P_T   (ones-augmented V gives softmax
                                             denominators as an extra row)
  att = att_T * (1/denom)         (reciprocal + DMA row-broadcast)
  out = att @ Wo^T + bo           (head-PAIR stacked K=128 matmuls,
                                   bo via a K=1 ones matmul)
"""

import numpy as np
import ml_dtypes

import concourse.bass as bass
import concourse.tile as tile
from concourse import bacc, mybir

FP32 = mybir.dt.float32
BF16 = mybir.dt.bfloat16
FP8 = mybir.dt.float8e4

N_CORES = 8
B, V, D, H, NHOP = 16, 1024, 512, 8, 9

BF16_NP = ml_dtypes.bfloat16

GATHER_FP8 = False
GDT = FP8 if GATHER_FP8 else BF16


class Cfg:
    def __init__(self, NC, B, V, D, H, NHOP):
        self.NC, self.B, self.V, self.D, self.H, self.NHOP = NC, B, V, D, H, NHOP
        assert B % NC == 0 and H == NC
        self.BPC = B // NC           # batches per core
        self.HD = D // H             # head dim
        self.T = self.BPC * V        # tokens per core
        assert D % 128 == 0 and V % 128 == 0
        self.DCH = D // 128          # contraction chunks for projections
        self.NJT = V // 128          # key-position tiles
        self.TCH = min(512, self.T)  # projection token chunk
        self.NTC = self.T // self.TCH
        self.NTT = self.T // 128     # token tiles
        assert self.HD == 64
        self.NG = self.H // 2        # head-pair groups


def build_graph(tc, outs, ins, cfg):
    """Emit the per-core graph. `ins` is a dict name->AP of DRAM inputs,
    `outs` a single DRAM AP [BPC, V, D] f32."""
    from contextlib import ExitStack

    ctx = ExitStack()
    nc = tc.nc
    c = cfg
    xT_d, WqT_d, WkT_d, WvT_d = ins["xT"], ins["WqT"], ins["WkT"], ins["WvT"]
    WoT_d, bo_d, rpeh_d, hopT_d = ins["WoT"], ins["bo"], ins["rpeh"], ins["hopT"]
    out_d = outs

    scale = 1.0 / float(np.sqrt(c.HD))

    consts = ctx.enter_context(tc.tile_pool(name="consts", bufs=1))
    persist = ctx.enter_context(tc.tile_pool(name="persist", bufs=1))
    dram = ctx.enter_context(tc.tile_pool(name="dram", bufs=1, space="DRAM"))
    dram2 = ctx.enter_context(tc.tile_pool(name="dram2", bufs=4, space="DRAM"))

    # ---- constants -------------------------------------------------------
    ones_col = consts.tile([1, 128], BF16, name="ones_col")
    nc.vector.memset(ones_col[:], 1.0)
    rpe_cols = consts.tile([128, c.NHOP], FP32, name="rpe_cols")
    nc.sync.dma_start(rpe_cols[:], rpeh_d.broadcast_to([128, c.NHOP]))
    # exp-space rpe: e_cols[p, m] = exp(rpe[head, m]) broadcast down partitions
    e_cols = consts.tile([128, c.NHOP], FP32, name="e_cols")
    nc.scalar.activation(e_cols[:], rpe_cols[:],
                         mybir.ActivationFunctionType.Exp)
    bo_f = consts.tile([1, c.D], FP32, name="bo_f")
    nc.sync.dma_start(bo_f[:], bo_d)
    bo_bf = consts.tile([1, c.D], BF16, name="bo_bf")
    nc.vector.tensor_copy(bo_bf[:], bo_f[:])

    # ---- input staging (bf16 direct, no casts); ALL input DMAs are
    # emitted before any dependent DMA so the in-order sync queue never
    # stalls the projection inputs.
    ctx_w = ExitStack()
    wpool = ctx_w.enter_context(tc.tile_pool(name="wpool", bufs=1))
    hop_pool = ctx_w.enter_context(tc.tile_pool(name="hopp", bufs=1))

    xT = [wpool.tile([128, c.T], BF16, name=f"xT{k}") for k in range(c.DCH)]
    for k in range(c.DCH):
        nc.sync.dma_start(xT[k][:], xT_d[k * 128:(k + 1) * 128, :])

    hop_t = [hop_pool.tile([128, c.V], BF16, name=f"hop{jt}")
             for jt in range(c.NJT)]
    for jt in range(c.NJT):
        nc.sync.dma_start(hop_t[jt][:], hopT_d[jt * 128:(jt + 1) * 128, :])

    def load_w(d_ap, nm):
        w = [wpool.tile([128, c.D], BF16, name=f"{nm}{k}") for k in range(c.DCH)]
        for k in range(c.DCH):
            nc.sync.dma_start(w[k][:], d_ap[k * 128:(k + 1) * 128, :])
        return w

    WqT = load_w(WqT_d, "WqT")
    WkT = load_w(WkT_d, "WkT")
    WvT = load_w(WvT_d, "WvT")
    # WoT rows grouped per head-pair: WoP[g] = WoT[g*128:(g+1)*128, :]
    WoP = [persist.tile([128, c.D], BF16, name=f"WoP{g}") for g in range(c.NG)]
    for g in range(c.NG):
        nc.sync.dma_start(WoP[g][:], WoT_d[g * 128:(g + 1) * 128, :])

    # ---- exp-bias build (own head, VectorE) + AllGather halves -----------
    ctx_bias = ExitStack()
    bias_pools = ctx_bias.enter_context(tc.tile_pool(name="biasb", bufs=2))
    half = (c.NJT // 2) * 128
    jt_half = c.NJT // 2
    bias_local_h = [dram.tile([half, c.V], GDT, name=f"bias_local{i}")
                    for i in range(2)]
    bias_all_h = [dram.tile([c.H, half, c.V], GDT, name=f"bias_all{i}",
                            addr_space="Shared")
                  for i in range(2)]
    for jt in range(c.NJT):
        hop_b = hop_t[jt]
        acc = bias_pools.tile([128, c.V], BF16, name="bacc", tag="bacc")
        nc.vector.tensor_scalar(
            acc[:], hop_b[:], 0.0, e_cols[:, 0:1],
            mybir.AluOpType.is_equal, mybir.AluOpType.mult,
        )
        for m in range(1, c.NHOP):
            term = bias_pools.tile([128, c.V], BF16, name="bterm", tag="bterm")
            nc.vector.tensor_scalar(
                term[:], hop_b[:], float(m), e_cols[:, m:m + 1],
                mybir.AluOpType.is_equal, mybir.AluOpType.mult,
            )
            nc.vector.tensor_tensor(acc[:], acc[:], term[:],
                                    mybir.AluOpType.add)
        hi, jr = divmod(jt, jt_half)
        nc.sync.dma_start(
            bias_local_h[hi][jr * 128:(jr + 1) * 128, :], acc[:])
        if jr == jt_half - 1:
            nc.gpsimd.collective_compute(
                "AllGather",
                mybir.AluOpType.bypass,
                replica_groups=[list(range(c.NC))],
                ins=[bias_local_h[hi].opt()],
                outs=[bias_all_h[hi].opt()],
            )
    ctx_bias.close()

    # ---- projections (PE, ScalarE evacuation) ----------------------------
    ctx_proj = ExitStack()
    ps_proj = ctx_proj.enter_context(
        tc.tile_pool(name="ps_proj", bufs=3, space="PSUM"))

    qT = [persist.tile([128, c.T], BF16, name=f"qT{g}") for g in range(c.DCH)]
    kT = [persist.tile([128, c.T], BF16, name=f"kT{g}") for g in range(c.DCH)]
    for q in range(c.DCH):
        for dst, W, sc in ((qT, WqT, scale), (kT, WkT, 1.0)):
            for t in range(c.NTC):
                ps = ps_proj.tile([128, c.TCH], FP32, name="ps_p", tag="ps_p")
                for k in range(c.DCH):
                    nc.tensor.matmul(
                        ps[:], W[k][:, q * 128:(q + 1) * 128],
                        xT[k][:, t * c.TCH:(t + 1) * c.TCH],
                        start=(k == 0), stop=(k == c.DCH - 1),
                    )
                nc.scalar.activation(
                    dst[q][:, t * c.TCH:(t + 1) * c.TCH], ps[:],
                    mybir.ActivationFunctionType.Copy, scale=float(sc))

    # v in token layout, ones-augmented: vt[tt] = [128, H, HD+1]
    vt = [persist.tile([128, c.H, c.HD + 1], BF16, name=f"vt{tt}")
          for tt in range(c.NTT)]
    for tt in range(c.NTT):
        ps = ps_proj.tile([128, c.D], FP32, name="ps_v", tag="ps_v")
        for k in range(c.DCH):
            nc.tensor.matmul(
                ps[:], xT[k][:, tt * 128:(tt + 1) * 128], WvT[k][:],
                start=(k == 0), stop=(k == c.DCH - 1),
            )
        nc.scalar.activation(
            vt[tt][:, :, 0:c.HD],
            ps[:].rearrange("p (h d) -> p h d", h=c.H),
            mybir.ActivationFunctionType.Copy)
        nc.vector.memset(vt[tt][:, :, c.HD:c.HD + 1], 1.0)

    ctx_proj.close()
    ctx_w.close()

    # ---- attention core ---------------------------------------------------
    att_pool = ctx.enter_context(tc.tile_pool(name="attn", bufs=1))
    ctx_att = ExitStack()
    biast_pool = ctx_att.enter_context(tc.tile_pool(name="biast", bufs=8))
    p_pool = ctx_att.enter_context(tc.tile_pool(name="psb", bufs=14))
    rec_pool = ctx_att.enter_context(tc.tile_pool(name="rec", bufs=1))
    ps_s_pool = ctx_att.enter_context(
        tc.tile_pool(name="ps_s", bufs=1, space="PSUM"))
    ps_att_pool = ctx_att.enter_context(
        tc.tile_pool(name="ps_att", bufs=1, space="PSUM"))

    # bias tiles for group g are prefetched while group g-1 computes
    biast = {}

    def fetch_bias(g):
        hA = 2 * g
        for jt in range(c.NJT):
            hi, jr = divmod(jt, jt_half)
            bt = biast_pool.tile([128, 2, c.V], BF16, name="bt", tag="bt")
            nc.sync.dma_start(
                bt[:], bias_all_h[hi][hA:hA + 2,
                                      jr * 128:(jr + 1) * 128,
                                      :].rearrange("h p i -> p h i"))
            biast[(g, jt)] = bt

    fetch_bias(0)
    att_n = {}
    p_tiles = {}     # (g, b, jt) -> (pA, pB)
    ps_att_t = {}    # (g, b) -> (ps_attA, ps_attB)

    def emit_s(g, b, jt):
        t0 = b * c.V
        jsl = slice(t0 + jt * 128, t0 + (jt + 1) * 128)
        ps_sA = ps_s_pool.tile([128, c.V], FP32, name="ps_sA", tag="ps_sA")
        ps_sB = ps_s_pool.tile([128, c.V], FP32, name="ps_sB", tag="ps_sB")
        for sc in range(2):
            ssl = slice(t0 + sc * 512, t0 + (sc + 1) * 512)
            osl = slice(sc * 512, (sc + 1) * 512)
            nc.tensor.matmul(ps_sA[:, osl], kT[g][0:c.HD, jsl],
                             qT[g][0:c.HD, ssl], start=True, stop=True)
            nc.tensor.matmul(ps_sB[:, osl], kT[g][c.HD:128, jsl],
                             qT[g][c.HD:128, ssl], start=True, stop=True)
        p2 = p_pool.tile([128, 2, c.V], BF16, name="p2", tag="p2")
        nc.scalar.activation(p2[:, 0, :], ps_sA[:],
                             mybir.ActivationFunctionType.Exp)
        nc.scalar.activation(p2[:, 1, :], ps_sB[:],
                             mybir.ActivationFunctionType.Exp)
        nc.vector.tensor_tensor(p2[:], p2[:], biast[(g, jt)][:],
                                mybir.AluOpType.mult)
        p_tiles[(g, b, jt)] = p2

    def emit_pv(g, b, jt):
        if jt == 0:
            ps_att_t[(g, b)] = (
                ps_att_pool.tile([c.HD + 1, c.V], FP32, name="ps_aA",
                                 tag="ps_aA"),
                ps_att_pool.tile([c.HD + 1, c.V], FP32, name="ps_aB",
                                 tag="ps_aB"))
        ps_attA, ps_attB = ps_att_t[(g, b)]
        p2 = p_tiles[(g, b, jt)]
        for ic in range(2):
            isl = slice(ic * 512, (ic + 1) * 512)
            nc.tensor.matmul(
                ps_attA[:, isl], vt[b * c.NJT + jt][:, 2 * g, :],
                p2[:, 0, isl],
                start=(jt == 0), stop=(jt == c.NJT - 1))
            nc.tensor.matmul(
                ps_attB[:, isl], vt[b * c.NJT + jt][:, 2 * g + 1, :],
                p2[:, 1, isl],
                start=(jt == 0), stop=(jt == c.NJT - 1))

    def normalize(g, b):
        ps_attA, ps_attB = ps_att_t[(g, b)]
        at = att_pool.tile([128, c.V], BF16, name=f"at{g}_{b}")
        att_n[(g, b)] = at
        # evacuate PSUM att first: a single copy per head frees the PSUM
        # banks ~4us earlier than the full normalize chain, so the next
        # block's first PV (and the PE stream behind it) is not blocked
        araws = []
        for ps_att, dt in ((ps_attA, "A"), (ps_attB, "B")):
            araw = rec_pool.tile([c.HD + 1, c.V], FP32, name="araw",
                                 tag=f"araw{dt}")
            nc.vector.tensor_copy(araw[:], ps_att[:])
            araws.append(araw)
        for araw, rows, dt in ((araws[0], slice(0, c.HD), "A"),
                               (araws[1], slice(c.HD, 128), "B")):
            den = rec_pool.tile([1, c.V], FP32, name="den",
                                tag=f"den{dt}{(g + b) % 2}")
            nc.vector.tensor_copy(den[:], araw[c.HD:c.HD + 1, :])
            nc.vector.reciprocal_approx_fast(den[:], den[:])
            den_dram = dram2.tile([1, c.V], FP32, name="den_dram",
                                  tag=f"den_dram{dt}{(g + b) % 2}")
            nc.gpsimd.dma_start(den_dram[:], den[:])
            rec_bc = rec_pool.tile([c.HD, c.V], FP32, name="rec_bc",
                                   tag=f"rec_bc{dt}{(g + b) % 2}")
            nc.gpsimd.dma_start(
                rec_bc[:], den_dram[:].broadcast_to([c.HD, c.V]))
            nc.vector.tensor_tensor(at[rows, :], araw[0:c.HD, :],
                                    rec_bc[:], mybir.AluOpType.mult)

    # Global software pipeline over all (g, b, jt): S/exp/mult run `depth`
    # jt-iterations ahead of PV.  A deep early pipeline rides out the
    # AllGather latency (S and exp do not depend on the collective); a
    # shallow steady-state depth overlaps consecutive blocks so the
    # normalize/PSUM-free chain hides behind the next block's S stream.
    iters = [(g, b, jt) for g in range(c.NG) for b in range(c.BPC)
             for jt in range(c.NJT)]
    NIT = len(iters)

    def depth_for(pv_idx):
        if pv_idx < 8:
            return 13
        return 12 if iters[pv_idx][2] == 0 else 8

    s_ptr = pv_ptr = 0
    while pv_ptr < NIT:
        if s_ptr < NIT and (s_ptr - pv_ptr) < depth_for(pv_ptr):
            g, b, jt = iters[s_ptr]
            if jt == 0 and b == 1 and g + 1 < c.NG:
                fetch_bias(g + 1)
            emit_s(g, b, jt)
            s_ptr += 1
        else:
            g, b, jt = iters[pv_ptr]
            emit_pv(g, b, jt)
            pv_ptr += 1
            if jt == c.NJT - 1:
                normalize(g, b)

    # ---- output projection (head-pair stacked, K=128) ---------------------
    # Emitted inside the attention scope, with PSUM borrowed from the ps_s
    # tags: those banks free right after the last exp, so the PE rolls from
    # the final PVs straight into the out matmuls without waiting for the
    # last block's accumulator evacuation (keeps the HAM clock-gate warm).
    ctx_out = ExitStack()
    outsb_pool = ctx_out.enter_context(tc.tile_pool(name="outsb", bufs=2))
    for b in range(c.BPC):
        for tt in range(c.NJT):
            tagn = "ps_sA" if (b * c.NJT + tt) % 2 == 0 else "ps_sB"
            nm = "ps_sA" if tagn == "ps_sA" else "ps_sB"
            ps_o2 = ps_s_pool.tile([128, c.V], FP32, name=nm, tag=tagn)
            ps_o = ps_o2[:, 0:c.D]
            nc.tensor.matmul(ps_o, ones_col[:], bo_bf[:],
                             start=True, stop=False)
            for g in range(c.NG):
                nc.tensor.matmul(
                    ps_o,
                    att_n[(g, b)][:, tt * 128:(tt + 1) * 128],
                    WoP[g][:],
                    start=False, stop=(g == c.NG - 1),
                )
            o_sb = outsb_pool.tile([128, c.D], FP32, name="o_sb", tag="o_sb")
            nc.scalar.activation(o_sb[:], ps_o,
                                 mybir.ActivationFunctionType.Copy)
            nc.sync.dma_start(out_d[b, tt * 128:(tt + 1) * 128, :], o_sb[:])

    ctx_out.close()
    ctx_att.close()
    ctx.close()


# --------------------------------------------------------------------------
# Host side
# --------------------------------------------------------------------------

def shard_inputs(x, Wq, Wk, Wv, Wo, bo, rpe, hop_matrix, cfg):
    c = cfg
    WqT = np.ascontiguousarray(Wq.T).astype(BF16_NP)
    WkT = np.ascontiguousarray(Wk.T).astype(BF16_NP)
    WvT = np.ascontiguousarray(Wv.T).astype(BF16_NP)
    WoT = np.ascontiguousarray(Wo.T).astype(BF16_NP)
    hopT = np.ascontiguousarray(hop_matrix.T).astype(BF16_NP)
    bo2 = np.ascontiguousarray(bo.astype(np.float32).reshape(1, c.D))
    in_maps = []
    for core in range(c.NC):
        xs = x[core * c.BPC:(core + 1) * c.BPC].astype(np.float32)
        xT = np.ascontiguousarray(xs.reshape(c.T, c.D).T).astype(BF16_NP)
        rpe_c = rpe[core].astype(np.float64)
        if GATHER_FP8:
            # softmax is invariant to scaling a head's exp-bias by alpha;
            # pick alpha to align exp(rpe) with the fp8(e4m3) grid
            e = np.exp(rpe_c)
            cands = np.linspace(1.0, 2.0, 513)[:-1]
            best_a, best_err = 1.0, np.inf
            for a in cands:
                q = np.asarray(a * e, dtype=ml_dtypes.float8_e4m3)
                err = np.sqrt(np.mean((q.astype(np.float64) / (a * e) - 1.0) ** 2))
                if err < best_err:
                    best_a, best_err = a, err
            rpe_c = rpe_c + np.log(best_a)
        in_maps.append({
            "xT": xT, "WqT": WqT, "WkT": WkT, "WvT": WvT, "WoT": WoT,
            "bo": bo2, "rpeh": np.ascontiguousarray(
                rpe_c.reshape(1, -1).astype(np.float32)),
            "hopT": hopT,
        })
    return in_maps


_CACHE = {}


def _get_compiled(cfg):
    key = (cfg.NC, cfg.B, cfg.V, cfg.D, cfg.H, cfg.NHOP)
    if key in _CACHE:
        return _CACHE[key]
    c = cfg
    nc = bacc.Bacc("TRN2", target_bir_lowering=False, debug=False,
                   num_devices=c.NC)
    ins = {
        "xT": nc.dram_tensor("xT", [c.D, c.T], BF16, kind="ExternalInput").ap(),
        "WqT": nc.dram_tensor("WqT", [c.D, c.D], BF16, kind="ExternalInput").ap(),
        "WkT": nc.dram_tensor("WkT", [c.D, c.D], BF16, kind="ExternalInput").ap(),
        "WvT": nc.dram_tensor("WvT", [c.D, c.D], BF16, kind="ExternalInput").ap(),
        "WoT": nc.dram_tensor("WoT", [c.D, c.D], BF16, kind="ExternalInput").ap(),
        "bo": nc.dram_tensor("bo", [1, c.D], FP32, kind="ExternalInput").ap(),
        "rpeh": nc.dram_tensor("rpeh", [1, c.NHOP], FP32,
                               kind="ExternalInput").ap(),
        "hopT": nc.dram_tensor("hopT", [c.V, c.V], BF16,
                               kind="ExternalInput").ap(),
    }
    out = nc.dram_tensor("out", [c.BPC, c.V, c.D], FP32,
                         kind="ExternalOutput").ap()
    with tile.TileContext(nc) as tc:
        build_graph(tc, out, ins, cfg)
    nc.compile()
    _CACHE[key] = nc
    return nc


def kernel(x, Wq, Wk, Wv, Wo, bo, rpe, hop_matrix):
    from concourse.bass_utils import run_bass_kernel_spmd

    cfg = Cfg(N_CORES, B, V, D, H, NHOP)
    nc = _get_compiled(cfg)
    in_maps = shard_inputs(np.asarray(x), np.asarray(Wq), np.asarray(Wk),
                           np.asarray(Wv), np.asarray(Wo), np.asarray(bo),
                           np.asarray(rpe), np.asarray(hop_matrix), cfg)
    res = run_bass_kernel_spmd(nc, in_maps, core_ids=list(range(cfg.NC)))
    return np.concatenate([res.results[c]["out"] for c in range(cfg.NC)],
                          axis=0)


# revision 47
# speedup vs baseline: 1.1044x; 1.1044x over previous
"""Trainium2 Bass kernel for nn_AttentionLayer (B=16, V=1024, D=512, H=8, MAXHOP=8).

Sharding: data-parallel over batch B across 8 NeuronCores (2 batches/core).
The relative-position bias is applied in EXP SPACE: P = exp(S) * expB where
expB = exp(rpe)[hop].  Core c builds head c's expB table on-chip (9-pass
select-accumulate on the Vector engine), then two AllGather halves
distribute all 8 heads to every core (optionally in fp8 to halve the
collective volume - the table has only 9 distinct values per head, so fp8
costs one ~3% quantization of those values, no accumulation error).
Because the bias is multiplicative-after-exp, the S matmuls and the exp()
do NOT depend on the collective - only the elementwise multiply does, so
the PE/ScalarE stream runs ahead of the gather.

Per-core math (transposed-score layout, softmax on native axes):
  qT/kT = (W @ x^T) per head      [HD, tokens]  (bf16, fp32 accum; host
                                   pre-casts x and weights to bf16)
  S_T[j,i] = k_j . q_i * scale    (2x N=512 matmuls per head-tile)
  E_T = exp(S_T)                  (ScalarE, PSUM -> SBUF bf16, one op/tile)
  P_T = E_T * expB_T              (VectorE)
  att_T[d,i] (+denom row) = [v|1]^T @ P_T   (ones-augmented V gives softmax
                                             denominators as an extra row)
  att = att_T * (1/denom)         (reciprocal + DMA row-broadcast)
  out = att @ Wo^T + bo           (head-PAIR stacked K=128 matmuls,
                                   bo via a K=1 ones matmul)
"""

import numpy as np
import ml_dtypes

import concourse.bass as bass
import concourse.tile as tile
from concourse import bacc, mybir

FP32 = mybir.dt.float32
BF16 = mybir.dt.bfloat16
FP8 = mybir.dt.float8e4

N_CORES = 8
B, V, D, H, NHOP = 16, 1024, 512, 8, 9

BF16_NP = ml_dtypes.bfloat16

GATHER_FP8 = False
GDT = FP8 if GATHER_FP8 else BF16


class Cfg:
    def __init__(self, NC, B, V, D, H, NHOP):
        self.NC, self.B, self.V, self.D, self.H, self.NHOP = NC, B, V, D, H, NHOP
        assert B % NC == 0 and H == NC
        self.BPC = B // NC           # batches per core
        self.HD = D // H             # head dim
        self.T = self.BPC * V        # tokens per core
        assert D % 128 == 0 and V % 128 == 0
        self.DCH = D // 128          # contraction chunks for projections
        self.NJT = V // 128          # key-position tiles
        self.TCH = min(512, self.T)  # projection token chunk
        self.NTC = self.T // self.TCH
        self.NTT = self.T // 128     # token tiles
        assert self.HD == 64
        self.NG = self.H // 2        # head-pair groups


def build_graph(tc, outs, ins, cfg):
    """Emit the per-core graph. `ins` is a dict name->AP of DRAM inputs,
    `outs` a single DRAM AP [BPC, V, D] f32."""
    from contextlib import ExitStack

    ctx = ExitStack()
    nc = tc.nc
    c = cfg
    xT_d, WqT_d, WkT_d, WvT_d = ins["xT"], ins["WqT"], ins["WkT"], ins["WvT"]
    WoT_d, bo_d, rpeh_d, hopT_d = ins["WoT"], ins["bo"], ins["rpeh"], ins["hopT"]
    out_d = outs

    scale = 1.0 / float(np.sqrt(c.HD))

    consts = ctx.enter_context(tc.tile_pool(name="consts", bufs=1))
    persist = ctx.enter_context(tc.tile_pool(name="persist", bufs=1))
    dram = ctx.enter_context(tc.tile_pool(name="dram", bufs=1, space="DRAM"))
    dram2 = ctx.enter_context(tc.tile_pool(name="dram2", bufs=4, space="DRAM"))

    # ---- constants -------------------------------------------------------
    ones_col = consts.tile([1, 128], BF16, name="ones_col")
    nc.vector.memset(ones_col[:], 1.0)
    rpe_cols = consts.tile([128, c.NHOP], FP32, name="rpe_cols")
    nc.sync.dma_start(rpe_cols[:], rpeh_d.broadcast_to([128, c.NHOP]))
    # exp-space rpe: e_cols[p, m] = exp(rpe[head, m]) broadcast down partitions
    e_cols = consts.tile([128, c.NHOP], FP32, name="e_cols")
    nc.scalar.activation(e_cols[:], rpe_cols[:],
                         mybir.ActivationFunctionType.Exp)
    bo_f = consts.tile([1, c.D], FP32, name="bo_f")
    nc.sync.dma_start(bo_f[:], bo_d)
    bo_bf = consts.tile([1, c.D], BF16, name="bo_bf")
    nc.vector.tensor_copy(bo_bf[:], bo_f[:])

    # ---- input staging (bf16 direct, no casts); ALL input DMAs are
    # emitted before any dependent DMA so the in-order sync queue never
    # stalls the projection inputs.
    ctx_w = ExitStack()
    wpool = ctx_w.enter_context(tc.tile_pool(name="wpool", bufs=1))
    hop_pool = ctx_w.enter_context(tc.tile_pool(name="hopp", bufs=1))

    xT = [wpool.tile([128, c.T], BF16, name=f"xT{k}") for k in range(c.DCH)]
    for k in range(c.DCH):
        nc.sync.dma_start(xT[k][:], xT_d[k * 128:(k + 1) * 128, :])

    hop_t = [hop_pool.tile([128, c.V], BF16, name=f"hop{jt}")
             for jt in range(c.NJT)]
    for jt in range(c.NJT):
        nc.sync.dma_start(hop_t[jt][:], hopT_d[jt * 128:(jt + 1) * 128, :])

    def load_w(d_ap, nm):
        w = [wpool.tile([128, c.D], BF16, name=f"{nm}{k}") for k in range(c.DCH)]
        for k in range(c.DCH):
            nc.sync.dma_start(w[k][:], d_ap[k * 128:(k + 1) * 128, :])
        return w

    WqT = load_w(WqT_d, "WqT")
    WkT = load_w(WkT_d, "WkT")
    WvT = load_w(WvT_d, "WvT")
    # WoT rows grouped per head-pair: WoP[g] = WoT[g*128:(g+1)*128, :]
    WoP = [persist.tile([128, c.D], BF16, name=f"WoP{g}") for g in range(c.NG)]
    for g in range(c.NG):
        nc.sync.dma_start(WoP[g][:], WoT_d[g * 128:(g + 1) * 128, :])

    # ---- exp-bias build (own head, VectorE) + AllGather halves -----------
    ctx_bias = ExitStack()
    bias_pools = ctx_bias.enter_context(tc.tile_pool(name="biasb", bufs=2))
    half = (c.NJT // 2) * 128
    jt_half = c.NJT // 2
    bias_local_h = [dram.tile([half, c.V], GDT, name=f"bias_local{i}")
                    for i in range(2)]
    bias_all_h = [dram.tile([c.H, half, c.V], GDT, name=f"bias_all{i}",
                            addr_space="Shared")
                  for i in range(2)]
    for jt in range(c.NJT):
        hop_b = hop_t[jt]
        acc = bias_pools.tile([128, c.V], BF16, name="bacc", tag="bacc")
        nc.vector.tensor_scalar(
            acc[:], hop_b[:], 0.0, e_cols[:, 0:1],
            mybir.AluOpType.is_equal, mybir.AluOpType.mult,
        )
        for m in range(1, c.NHOP):
            term = bias_pools.tile([128, c.V], BF16, name="bterm", tag="bterm")
            nc.vector.tensor_scalar(
                term[:], hop_b[:], float(m), e_cols[:, m:m + 1],
                mybir.AluOpType.is_equal, mybir.AluOpType.mult,
            )
            nc.vector.tensor_tensor(acc[:], acc[:], term[:],
                                    mybir.AluOpType.add)
        hi, jr = divmod(jt, jt_half)
        nc.sync.dma_start(
            bias_local_h[hi][jr * 128:(jr + 1) * 128, :], acc[:])
        if jr == jt_half - 1:
            nc.gpsimd.collective_compute(
                "AllGather",
                mybir.AluOpType.bypass,
                replica_groups=[list(range(c.NC))],
                ins=[bias_local_h[hi].opt()],
                outs=[bias_all_h[hi].opt()],
            )
    ctx_bias.close()

    # ---- projections (PE, ScalarE evacuation) ----------------------------
    ctx_proj = ExitStack()
    ps_proj = ctx_proj.enter_context(
        tc.tile_pool(name="ps_proj", bufs=3, space="PSUM"))

    qT = [persist.tile([128, c.T], BF16, name=f"qT{g}") for g in range(c.DCH)]
    kT = [persist.tile([128, c.T], BF16, name=f"kT{g}") for g in range(c.DCH)]
    for q in range(c.DCH):
        for dst, W, sc in ((qT, WqT, scale), (kT, WkT, 1.0)):
            for t in range(c.NTC):
                ps = ps_proj.tile([128, c.TCH], FP32, name="ps_p", tag="ps_p")
                for k in range(c.DCH):
                    nc.tensor.matmul(
                        ps[:], W[k][:, q * 128:(q + 1) * 128],
                        xT[k][:, t * c.TCH:(t + 1) * c.TCH],
                        start=(k == 0), stop=(k == c.DCH - 1),
                    )
                nc.scalar.activation(
                    dst[q][:, t * c.TCH:(t + 1) * c.TCH], ps[:],
                    mybir.ActivationFunctionType.Copy, scale=float(sc))

    # v in token layout, ones-augmented: vt[tt] = [128, H, HD+1]
    vt = [persist.tile([128, c.H, c.HD + 1], BF16, name=f"vt{tt}")
          for tt in range(c.NTT)]
    for tt in range(c.NTT):
        ps = ps_proj.tile([128, c.D], FP32, name="ps_v", tag="ps_v")
        for k in range(c.DCH):
            nc.tensor.matmul(
                ps[:], xT[k][:, tt * 128:(tt + 1) * 128], WvT[k][:],
                start=(k == 0), stop=(k == c.DCH - 1),
            )
        nc.scalar.activation(
            vt[tt][:, :, 0:c.HD],
            ps[:].rearrange("p (h d) -> p h d", h=c.H),
            mybir.ActivationFunctionType.Copy)
        nc.vector.memset(vt[tt][:, :, c.HD:c.HD + 1], 1.0)

    ctx_proj.close()
    ctx_w.close()

    # ---- attention core ---------------------------------------------------
    att_pool = ctx.enter_context(tc.tile_pool(name="attn", bufs=1))
    ctx_att = ExitStack()
    biast_pool = ctx_att.enter_context(tc.tile_pool(name="biast", bufs=9))
    p_pool = ctx_att.enter_context(tc.tile_pool(name="psb", bufs=13))
    rec_pool = ctx_att.enter_context(tc.tile_pool(name="rec", bufs=1))
    ps_s_pool = ctx_att.enter_context(
        tc.tile_pool(name="ps_s", bufs=1, space="PSUM"))
    ps_att_pool = ctx_att.enter_context(
        tc.tile_pool(name="ps_att", bufs=1, space="PSUM"))

    # bias tiles for group g are prefetched while group g-1 computes
    biast = {}

    def fetch_bias(g):
        hA = 2 * g
        for jt in range(c.NJT):
            hi, jr = divmod(jt, jt_half)
            bt = biast_pool.tile([128, 2, c.V], BF16, name="bt", tag="bt")
            nc.sync.dma_start(
                bt[:], bias_all_h[hi][hA:hA + 2,
                                      jr * 128:(jr + 1) * 128,
                                      :].rearrange("h p i -> p h i"))
            biast[(g, jt)] = bt

    fetch_bias(0)
    att_n = {}
    p_tiles = {}     # (g, b, jt) -> (pA, pB)
    ps_att_t = {}    # (g, b) -> (ps_attA, ps_attB)

    def emit_s(g, b, jt):
        t0 = b * c.V
        jsl = slice(t0 + jt * 128, t0 + (jt + 1) * 128)
        ps_sA = ps_s_pool.tile([128, c.V], FP32, name="ps_sA", tag="ps_sA")
        ps_sB = ps_s_pool.tile([128, c.V], FP32, name="ps_sB", tag="ps_sB")
        for sc in range(2):
            ssl = slice(t0 + sc * 512, t0 + (sc + 1) * 512)
            osl = slice(sc * 512, (sc + 1) * 512)
            nc.tensor.matmul(ps_sA[:, osl], kT[g][0:c.HD, jsl],
                             qT[g][0:c.HD, ssl], start=True, stop=True)
            nc.tensor.matmul(ps_sB[:, osl], kT[g][c.HD:128, jsl],
                             qT[g][c.HD:128, ssl], start=True, stop=True)
        p2 = p_pool.tile([128, 2, c.V], BF16, name="p2", tag="p2")
        nc.scalar.activation(p2[:, 0, :], ps_sA[:],
                             mybir.ActivationFunctionType.Exp)
        nc.scalar.activation(p2[:, 1, :], ps_sB[:],
                             mybir.ActivationFunctionType.Exp)
        nc.vector.tensor_tensor(p2[:], p2[:], biast[(g, jt)][:],
                                mybir.AluOpType.mult)
        p_tiles[(g, b, jt)] = p2

    def emit_pv(g, b, jt):
        if jt == 0:
            ps_att_t[(g, b)] = (
                ps_att_pool.tile([c.HD + 1, c.V], FP32, name="ps_aA",
                                 tag="ps_aA"),
                ps_att_pool.tile([c.HD + 1, c.V], FP32, name="ps_aB",
                                 tag="ps_aB"))
        ps_attA, ps_attB = ps_att_t[(g, b)]
        p2 = p_tiles[(g, b, jt)]
        for ic in range(2):
            isl = slice(ic * 512, (ic + 1) * 512)
            nc.tensor.matmul(
                ps_attA[:, isl], vt[b * c.NJT + jt][:, 2 * g, :],
                p2[:, 0, isl],
                start=(jt == 0), stop=(jt == c.NJT - 1))
            nc.tensor.matmul(
                ps_attB[:, isl], vt[b * c.NJT + jt][:, 2 * g + 1, :],
                p2[:, 1, isl],
                start=(jt == 0), stop=(jt == c.NJT - 1))

    def normalize(g, b):
        ps_attA, ps_attB = ps_att_t[(g, b)]
        at = att_pool.tile([128, c.V], BF16, name=f"at{g}_{b}")
        att_n[(g, b)] = at
        # evacuate PSUM att first: a single copy per head frees the PSUM
        # banks ~4us earlier than the full normalize chain, so the next
        # block's first PV (and the PE stream behind it) is not blocked
        araws = []
        for ps_att, dt in ((ps_attA, "A"), (ps_attB, "B")):
            araw = rec_pool.tile([c.HD + 1, c.V], FP32, name="araw",
                                 tag=f"araw{dt}")
            nc.vector.tensor_copy(araw[:], ps_att[:])
            araws.append(araw)
        for araw, rows, dt in ((araws[0], slice(0, c.HD), "A"),
                               (araws[1], slice(c.HD, 128), "B")):
            den = rec_pool.tile([1, c.V], FP32, name="den",
                                tag=f"den{dt}{(g + b) % 2}")
            nc.vector.tensor_copy(den[:], araw[c.HD:c.HD + 1, :])
            nc.vector.reciprocal_approx_fast(den[:], den[:])
            den_dram = dram2.tile([1, c.V], FP32, name="den_dram",
                                  tag=f"den_dram{dt}{(g + b) % 2}")
            nc.gpsimd.dma_start(den_dram[:], den[:])
            rec_bc = rec_pool.tile([c.HD, c.V], FP32, name="rec_bc",
                                   tag=f"rec_bc{dt}{(g + b) % 2}")
            nc.gpsimd.dma_start(
                rec_bc[:], den_dram[:].broadcast_to([c.HD, c.V]))
            nc.vector.tensor_tensor(at[rows, :], araw[0:c.HD, :],
                                    rec_bc[:], mybir.AluOpType.mult)

    # Global software pipeline over all (g, b, jt): S/exp/mult run `depth`
    # jt-iterations ahead of PV.  A deep early pipeline rides out the
    # AllGather latency (S and exp do not depend on the collective); a
    # shallow steady-state depth overlaps consecutive blocks so the
    # normalize/PSUM-free chain hides behind the next block's S stream.
    iters = [(g, b, jt) for g in range(c.NG) for b in range(c.BPC)
             for jt in range(c.NJT)]
    NIT = len(iters)

    def depth_for(pv_idx):
        if pv_idx < 8:
            return 12
        return 12 if iters[pv_idx][2] == 0 else 8

    s_ptr = pv_ptr = 0
    while pv_ptr < NIT:
        if s_ptr < NIT and (s_ptr - pv_ptr) < depth_for(pv_ptr):
            g, b, jt = iters[s_ptr]
            if jt == 0 and b == 1 and g + 1 < c.NG:
                fetch_bias(g + 1)
            emit_s(g, b, jt)
            s_ptr += 1
        else:
            g, b, jt = iters[pv_ptr]
            emit_pv(g, b, jt)
            pv_ptr += 1
            if jt == c.NJT - 1:
                normalize(g, b)

    # ---- output projection (head-pair stacked, K=128) ---------------------
    # Emitted inside the attention scope, with PSUM borrowed from the ps_s
    # tags: those banks free right after the last exp, so the PE rolls from
    # the final PVs straight into the out matmuls without waiting for the
    # last block's accumulator evacuation (keeps the HAM clock-gate warm).
    ctx_out = ExitStack()
    outsb_pool = ctx_out.enter_context(tc.tile_pool(name="outsb", bufs=2))
    for b in range(c.BPC):
        for tt in range(c.NJT):
            tagn = "ps_sA" if (b * c.NJT + tt) % 2 == 0 else "ps_sB"
            nm = "ps_sA" if tagn == "ps_sA" else "ps_sB"
            ps_o2 = ps_s_pool.tile([128, c.V], FP32, name=nm, tag=tagn)
            ps_o = ps_o2[:, 0:c.D]
            nc.tensor.matmul(ps_o, ones_col[:], bo_bf[:],
                             start=True, stop=False)
            for g in range(c.NG):
                nc.tensor.matmul(
                    ps_o,
                    att_n[(g, b)][:, tt * 128:(tt + 1) * 128],
                    WoP[g][:],
                    start=False, stop=(g == c.NG - 1),
                )
            o_sb = outsb_pool.tile([128, c.D], FP32, name="o_sb", tag="o_sb")
            nc.scalar.activation(o_sb[:], ps_o,
                                 mybir.ActivationFunctionType.Copy)
            nc.sync.dma_start(out_d[b, tt * 128:(tt + 1) * 128, :], o_sb[:])

    ctx_out.close()
    ctx_att.close()
    ctx.close()


# --------------------------------------------------------------------------
# Host side
# --------------------------------------------------------------------------

def shard_inputs(x, Wq, Wk, Wv, Wo, bo, rpe, hop_matrix, cfg):
    c = cfg
    WqT = np.ascontiguousarray(Wq.T).astype(BF16_NP)
    WkT = np.ascontiguousarray(Wk.T).astype(BF16_NP)
    WvT = np.ascontiguousarray(Wv.T).astype(BF16_NP)
    WoT = np.ascontiguousarray(Wo.T).astype(BF16_NP)
    hopT = np.ascontiguousarray(hop_matrix.T).astype(BF16_NP)
    bo2 = np.ascontiguousarray(bo.astype(np.float32).reshape(1, c.D))
    in_maps = []
    for core in range(c.NC):
        xs = x[core * c.BPC:(core + 1) * c.BPC].astype(np.float32)
        xT = np.ascontiguousarray(xs.reshape(c.T, c.D).T).astype(BF16_NP)
        rpe_c = rpe[core].astype(np.float64)
        if GATHER_FP8:
            # softmax is invariant to scaling a head's exp-bias by alpha;
            # pick alpha to align exp(rpe) with the fp8(e4m3) grid
            e = np.exp(rpe_c)
            cands = np.linspace(1.0, 2.0, 513)[:-1]
            best_a, best_err = 1.0, np.inf
            for a in cands:
                q = np.asarray(a * e, dtype=ml_dtypes.float8_e4m3)
                err = np.sqrt(np.mean((q.astype(np.float64) / (a * e) - 1.0) ** 2))
                if err < best_err:
                    best_a, best_err = a, err
            rpe_c = rpe_c + np.log(best_a)
        in_maps.append({
            "xT": xT, "WqT": WqT, "WkT": WkT, "WvT": WvT, "WoT": WoT,
            "bo": bo2, "rpeh": np.ascontiguousarray(
                rpe_c.reshape(1, -1).astype(np.float32)),
            "hopT": hopT,
        })
    return in_maps


_CACHE = {}


def _get_compiled(cfg):
    key = (cfg.NC, cfg.B, cfg.V, cfg.D, cfg.H, cfg.NHOP)
    if key in _CACHE:
        return _CACHE[key]
    c = cfg
    nc = bacc.Bacc("TRN2", target_bir_lowering=False, debug=False,
                   num_devices=c.NC)
    ins = {
        "xT": nc.dram_tensor("xT", [c.D, c.T], BF16, kind="ExternalInput").ap(),
        "WqT": nc.dram_tensor("WqT", [c.D, c.D], BF16, kind="ExternalInput").ap(),
        "WkT": nc.dram_tensor("WkT", [c.D, c.D], BF16, kind="ExternalInput").ap(),
        "WvT": nc.dram_tensor("WvT", [c.D, c.D], BF16, kind="ExternalInput").ap(),
        "WoT": nc.dram_tensor("WoT", [c.D, c.D], BF16, kind="ExternalInput").ap(),
        "bo": nc.dram_tensor("bo", [1, c.D], FP32, kind="ExternalInput").ap(),
        "rpeh": nc.dram_tensor("rpeh", [1, c.NHOP], FP32,
                               kind="ExternalInput").ap(),
        "hopT": nc.dram_tensor("hopT", [c.V, c.V], BF16,
                               kind="ExternalInput").ap(),
    }
    out = nc.dram_tensor("out", [c.BPC, c.V, c.D], FP32,
                         kind="ExternalOutput").ap()
    with tile.TileContext(nc) as tc:
        build_graph(tc, out, ins, cfg)
    nc.compile()
    _CACHE[key] = nc
    return nc


def kernel(x, Wq, Wk, Wv, Wo, bo, rpe, hop_matrix):
    from concourse.bass_utils import run_bass_kernel_spmd

    cfg = Cfg(N_CORES, B, V, D, H, NHOP)
    nc = _get_compiled(cfg)
    in_maps = shard_inputs(np.asarray(x), np.asarray(Wq), np.asarray(Wk),
                           np.asarray(Wv), np.asarray(Wo), np.asarray(bo),
                           np.asarray(rpe), np.asarray(hop_matrix), cfg)
    res = run_bass_kernel_spmd(nc, in_maps, core_ids=list(range(cfg.NC)))
    return np.concatenate([res.results[c]["out"] for c in range(cfg.NC)],
                          axis=0)


# revision 48
# speedup vs baseline: 1.1504x; 1.0417x over previous
"""Trainium2 Bass kernel for nn_AttentionLayer (B=16, V=1024, D=512, H=8, MAXHOP=8).

Sharding: data-parallel over batch B across 8 NeuronCores (2 batches/core).
The relative-position bias is applied in EXP SPACE: P = exp(S) * expB where
expB = exp(rpe)[hop].  Core c builds head c's expB table on-chip (9-pass
select-accumulate on the Vector engine), then two AllGather halves
distribute all 8 heads to every core (optionally in fp8 to halve the
collective volume - the table has only 9 distinct values per head, so fp8
costs one ~3% quantization of those values, no accumulation error).
Because the bias is multiplicative-after-exp, the S matmuls and the exp()
do NOT depend on the collective - only the elementwise multiply does, so
the PE/ScalarE stream runs ahead of the gather.

Per-core math (transposed-score layout, softmax on native axes):
  qT/kT = (W @ x^T) per head      [HD, tokens]  (bf16, fp32 accum; host
                                   pre-casts x and weights to bf16)
  S_T[j,i] = k_j . q_i * scale    (2x N=512 matmuls per head-tile)
  E_T = exp(S_T)                  (ScalarE, PSUM -> SBUF bf16, one op/tile)
  P_T = E_T * expB_T              (VectorE)
  att_T[d,i] (+denom row) = [v|1]^T @ P_T   (ones-augmented V gives softmax
                                             denominators as an extra row)
  att = att_T * (1/denom)         (reciprocal + DMA row-broadcast)
  out = att @ Wo^T + bo           (head-PAIR stacked K=128 matmuls,
                                   bo via a K=1 ones matmul)
"""

import numpy as np
import ml_dtypes

import concourse.bass as bass
import concourse.tile as tile
from concourse import bacc, mybir

FP32 = mybir.dt.float32
BF16 = mybir.dt.bfloat16
FP8 = mybir.dt.float8e4

N_CORES = 8
B, V, D, H, NHOP = 16, 1024, 512, 8, 9

BF16_NP = ml_dtypes.bfloat16

GATHER_FP8 = False
GDT = FP8 if GATHER_FP8 else BF16


class Cfg:
    def __init__(self, NC, B, V, D, H, NHOP):
        self.NC, self.B, self.V, self.D, self.H, self.NHOP = NC, B, V, D, H, NHOP
        assert B % NC == 0 and H == NC
        self.BPC = B // NC           # batches per core
        self.HD = D // H             # head dim
        self.T = self.BPC * V        # tokens per core
        assert D % 128 == 0 and V % 128 == 0
        self.DCH = D // 128          # contraction chunks for projections
        self.NJT = V // 128          # key-position tiles
        self.TCH = min(512, self.T)  # projection token chunk
        self.NTC = self.T // self.TCH
        self.NTT = self.T // 128     # token tiles
        assert self.HD == 64
        self.NG = self.H // 2        # head-pair groups


def build_graph(tc, outs, ins, cfg):
    """Emit the per-core graph. `ins` is a dict name->AP of DRAM inputs,
    `outs` a single DRAM AP [BPC, V, D] f32."""
    from contextlib import ExitStack

    ctx = ExitStack()
    nc = tc.nc
    c = cfg
    xT_d, WqT_d, WkT_d, WvT_d = ins["xT"], ins["WqT"], ins["WkT"], ins["WvT"]
    WoT_d, bo_d, rpeh_d, hopT_d = ins["WoT"], ins["bo"], ins["rpeh"], ins["hopT"]
    out_d = outs

    scale = 1.0 / float(np.sqrt(c.HD))

    consts = ctx.enter_context(tc.tile_pool(name="consts", bufs=1))
    persist = ctx.enter_context(tc.tile_pool(name="persist", bufs=1))
    dram = ctx.enter_context(tc.tile_pool(name="dram", bufs=1, space="DRAM"))
    dram2 = ctx.enter_context(tc.tile_pool(name="dram2", bufs=4, space="DRAM"))

    # ---- constants -------------------------------------------------------
    ones_col = consts.tile([1, 128], BF16, name="ones_col")
    nc.vector.memset(ones_col[:], 1.0)
    rpe_cols = consts.tile([128, c.NHOP], FP32, name="rpe_cols")
    nc.sync.dma_start(rpe_cols[:], rpeh_d.broadcast_to([128, c.NHOP]))
    # exp-space rpe: e_cols[p, m] = exp(rpe[head, m]) broadcast down partitions
    e_cols = consts.tile([128, c.NHOP], FP32, name="e_cols")
    nc.scalar.activation(e_cols[:], rpe_cols[:],
                         mybir.ActivationFunctionType.Exp)
    bo_f = consts.tile([1, c.D], FP32, name="bo_f")
    nc.sync.dma_start(bo_f[:], bo_d)
    bo_bf = consts.tile([1, c.D], BF16, name="bo_bf")
    nc.vector.tensor_copy(bo_bf[:], bo_f[:])

    # ---- input staging (bf16 direct, no casts); ALL input DMAs are
    # emitted before any dependent DMA so the in-order sync queue never
    # stalls the projection inputs.
    ctx_w = ExitStack()
    wpool = ctx_w.enter_context(tc.tile_pool(name="wpool", bufs=1))
    hop_pool = ctx_w.enter_context(tc.tile_pool(name="hopp", bufs=1))

    xT = [wpool.tile([128, c.T], BF16, name=f"xT{k}") for k in range(c.DCH)]
    for k in range(c.DCH):
        nc.sync.dma_start(xT[k][:], xT_d[k * 128:(k + 1) * 128, :])

    hop_t = [hop_pool.tile([128, c.V], BF16, name=f"hop{jt}")
             for jt in range(c.NJT)]
    for jt in range(c.NJT):
        nc.sync.dma_start(hop_t[jt][:], hopT_d[jt * 128:(jt + 1) * 128, :])

    def load_w(d_ap, nm):
        w = [wpool.tile([128, c.D], BF16, name=f"{nm}{k}") for k in range(c.DCH)]
        for k in range(c.DCH):
            nc.sync.dma_start(w[k][:], d_ap[k * 128:(k + 1) * 128, :])
        return w

    WqT = load_w(WqT_d, "WqT")
    WkT = load_w(WkT_d, "WkT")
    WvT = load_w(WvT_d, "WvT")
    # WoT rows grouped per head-pair: WoP[g] = WoT[g*128:(g+1)*128, :]
    WoP = [persist.tile([128, c.D], BF16, name=f"WoP{g}") for g in range(c.NG)]
    for g in range(c.NG):
        nc.sync.dma_start(WoP[g][:], WoT_d[g * 128:(g + 1) * 128, :])

    # ---- exp-bias build (own head, VectorE) + AllGather halves -----------
    ctx_bias = ExitStack()
    bias_pools = ctx_bias.enter_context(tc.tile_pool(name="biasb", bufs=2))
    half = (c.NJT // 2) * 128
    jt_half = c.NJT // 2
    bias_local_h = [dram.tile([half, c.V], GDT, name=f"bias_local{i}")
                    for i in range(2)]
    bias_all_h = [dram.tile([c.H, half, c.V], GDT, name=f"bias_all{i}",
                            addr_space="Shared")
                  for i in range(2)]
    for jt in range(c.NJT):
        hop_b = hop_t[jt]
        acc = bias_pools.tile([128, c.V], BF16, name="bacc", tag="bacc")
        nc.vector.tensor_scalar(
            acc[:], hop_b[:], 0.0, e_cols[:, 0:1],
            mybir.AluOpType.is_equal, mybir.AluOpType.mult,
        )
        for m in range(1, c.NHOP):
            term = bias_pools.tile([128, c.V], BF16, name="bterm", tag="bterm")
            nc.vector.tensor_scalar(
                term[:], hop_b[:], float(m), e_cols[:, m:m + 1],
                mybir.AluOpType.is_equal, mybir.AluOpType.mult,
            )
            nc.vector.tensor_tensor(acc[:], acc[:], term[:],
                                    mybir.AluOpType.add)
        hi, jr = divmod(jt, jt_half)
        nc.sync.dma_start(
            bias_local_h[hi][jr * 128:(jr + 1) * 128, :], acc[:])
        if jr == jt_half - 1:
            nc.gpsimd.collective_compute(
                "AllGather",
                mybir.AluOpType.bypass,
                replica_groups=[list(range(c.NC))],
                ins=[bias_local_h[hi].opt()],
                outs=[bias_all_h[hi].opt()],
            )
    ctx_bias.close()

    # ---- projections (PE, ScalarE evacuation) ----------------------------
    ctx_proj = ExitStack()
    ps_proj = ctx_proj.enter_context(
        tc.tile_pool(name="ps_proj", bufs=3, space="PSUM"))

    qT = [persist.tile([128, c.T], BF16, name=f"qT{g}") for g in range(c.DCH)]
    kT = [persist.tile([128, c.T], BF16, name=f"kT{g}") for g in range(c.DCH)]
    for q in range(c.DCH):
        for dst, W, sc in ((qT, WqT, scale), (kT, WkT, 1.0)):
            for t in range(c.NTC):
                ps = ps_proj.tile([128, c.TCH], FP32, name="ps_p", tag="ps_p")
                for k in range(c.DCH):
                    nc.tensor.matmul(
                        ps[:], W[k][:, q * 128:(q + 1) * 128],
                        xT[k][:, t * c.TCH:(t + 1) * c.TCH],
                        start=(k == 0), stop=(k == c.DCH - 1),
                    )
                nc.scalar.activation(
                    dst[q][:, t * c.TCH:(t + 1) * c.TCH], ps[:],
                    mybir.ActivationFunctionType.Copy, scale=float(sc))

    # v in token layout, ones-augmented: vt[tt] = [128, H, HD+1]
    vt = [persist.tile([128, c.H, c.HD + 1], BF16, name=f"vt{tt}")
          for tt in range(c.NTT)]
    for tt in range(c.NTT):
        ps = ps_proj.tile([128, c.D], FP32, name="ps_v", tag="ps_v")
        for k in range(c.DCH):
            nc.tensor.matmul(
                ps[:], xT[k][:, tt * 128:(tt + 1) * 128], WvT[k][:],
                start=(k == 0), stop=(k == c.DCH - 1),
            )
        nc.scalar.activation(
            vt[tt][:, :, 0:c.HD],
            ps[:].rearrange("p (h d) -> p h d", h=c.H),
            mybir.ActivationFunctionType.Copy)
        nc.vector.memset(vt[tt][:, :, c.HD:c.HD + 1], 1.0)

    ctx_proj.close()
    ctx_w.close()

    # ---- attention core ---------------------------------------------------
    att_pool = ctx.enter_context(tc.tile_pool(name="attn", bufs=1))
    ctx_att = ExitStack()
    biast_pool = ctx_att.enter_context(tc.tile_pool(name="biast", bufs=9))
    p_pool = ctx_att.enter_context(tc.tile_pool(name="psb", bufs=14))
    rec_pool = ctx_att.enter_context(tc.tile_pool(name="rec", bufs=1))
    ps_s_pool = ctx_att.enter_context(
        tc.tile_pool(name="ps_s", bufs=1, space="PSUM"))
    ps_att_pool = ctx_att.enter_context(
        tc.tile_pool(name="ps_att", bufs=1, space="PSUM"))

    # bias tiles for group g are prefetched while group g-1 computes
    biast = {}

    def fetch_bias(g):
        hA = 2 * g
        for jt in range(c.NJT):
            hi, jr = divmod(jt, jt_half)
            bt = biast_pool.tile([128, 2, c.V], BF16, name="bt", tag="bt")
            nc.sync.dma_start(
                bt[:], bias_all_h[hi][hA:hA + 2,
                                      jr * 128:(jr + 1) * 128,
                                      :].rearrange("h p i -> p h i"))
            biast[(g, jt)] = bt

    fetch_bias(0)
    att_n = {}
    p_tiles = {}     # (g, b, jt) -> (pA, pB)
    ps_att_t = {}    # (g, b) -> (ps_attA, ps_attB)

    def emit_s(g, b, jt):
        t0 = b * c.V
        jsl = slice(t0 + jt * 128, t0 + (jt + 1) * 128)
        ps_sA = ps_s_pool.tile([128, c.V], FP32, name="ps_sA", tag="ps_sA")
        ps_sB = ps_s_pool.tile([128, c.V], FP32, name="ps_sB", tag="ps_sB")
        for sc in range(2):
            ssl = slice(t0 + sc * 512, t0 + (sc + 1) * 512)
            osl = slice(sc * 512, (sc + 1) * 512)
            nc.tensor.matmul(ps_sA[:, osl], kT[g][0:c.HD, jsl],
                             qT[g][0:c.HD, ssl], start=True, stop=True)
            nc.tensor.matmul(ps_sB[:, osl], kT[g][c.HD:128, jsl],
                             qT[g][c.HD:128, ssl], start=True, stop=True)
        p2 = p_pool.tile([128, 2, c.V], BF16, name="p2", tag="p2")
        nc.scalar.activation(p2[:, 0, :], ps_sA[:],
                             mybir.ActivationFunctionType.Exp)
        nc.scalar.activation(p2[:, 1, :], ps_sB[:],
                             mybir.ActivationFunctionType.Exp)
        nc.vector.tensor_tensor(p2[:], p2[:], biast[(g, jt)][:],
                                mybir.AluOpType.mult)
        p_tiles[(g, b, jt)] = p2

    def emit_pv(g, b, jt):
        if jt == 0:
            ps_att_t[(g, b)] = (
                ps_att_pool.tile([c.HD + 1, c.V], FP32, name="ps_aA",
                                 tag="ps_aA"),
                ps_att_pool.tile([c.HD + 1, c.V], FP32, name="ps_aB",
                                 tag="ps_aB"))
        ps_attA, ps_attB = ps_att_t[(g, b)]
        p2 = p_tiles[(g, b, jt)]
        for ic in range(2):
            isl = slice(ic * 512, (ic + 1) * 512)
            nc.tensor.matmul(
                ps_attA[:, isl], vt[b * c.NJT + jt][:, 2 * g, :],
                p2[:, 0, isl],
                start=(jt == 0), stop=(jt == c.NJT - 1))
            nc.tensor.matmul(
                ps_attB[:, isl], vt[b * c.NJT + jt][:, 2 * g + 1, :],
                p2[:, 1, isl],
                start=(jt == 0), stop=(jt == c.NJT - 1))

    def normalize(g, b):
        ps_attA, ps_attB = ps_att_t[(g, b)]
        at = att_pool.tile([128, c.V], BF16, name=f"at{g}_{b}")
        att_n[(g, b)] = at
        # evacuate PSUM att first: a single copy per head frees the PSUM
        # banks ~4us earlier than the full normalize chain, so the next
        # block's first PV (and the PE stream behind it) is not blocked
        araws = []
        for ps_att, dt in ((ps_attA, "A"), (ps_attB, "B")):
            araw = rec_pool.tile([c.HD + 1, c.V], BF16, name="araw",
                                 tag=f"araw{dt}")
            nc.vector.tensor_copy(araw[:], ps_att[:])
            araws.append(araw)
        for araw, rows, dt in ((araws[0], slice(0, c.HD), "A"),
                               (araws[1], slice(c.HD, 128), "B")):
            den = rec_pool.tile([1, c.V], FP32, name="den",
                                tag=f"den{dt}{(g + b) % 2}")
            nc.vector.tensor_copy(den[:], araw[c.HD:c.HD + 1, :])
            nc.vector.reciprocal_approx_fast(den[:], den[:])
            den_dram = dram2.tile([1, c.V], FP32, name="den_dram",
                                  tag=f"den_dram{dt}{(g + b) % 2}")
            nc.gpsimd.dma_start(den_dram[:], den[:])
            rec_bc = rec_pool.tile([c.HD, c.V], BF16, name="rec_bc",
                                   tag=f"rec_bc{dt}{(g + b) % 2}")
            nc.gpsimd.dma_start(
                rec_bc[:], den_dram[:].broadcast_to([c.HD, c.V]))
            nc.vector.tensor_tensor(at[rows, :], araw[0:c.HD, :],
                                    rec_bc[:], mybir.AluOpType.mult)

    # Global software pipeline over all (g, b, jt): S/exp/mult run `depth`
    # jt-iterations ahead of PV.  A deep early pipeline rides out the
    # AllGather latency (S and exp do not depend on the collective); a
    # shallow steady-state depth overlaps consecutive blocks so the
    # normalize/PSUM-free chain hides behind the next block's S stream.
    iters = [(g, b, jt) for g in range(c.NG) for b in range(c.BPC)
             for jt in range(c.NJT)]
    NIT = len(iters)

    def depth_for(pv_idx):
        if pv_idx < 8:
            return 13
        return 12 if iters[pv_idx][2] == 0 else 8

    s_ptr = pv_ptr = 0
    while pv_ptr < NIT:
        if s_ptr < NIT and (s_ptr - pv_ptr) < depth_for(pv_ptr):
            g, b, jt = iters[s_ptr]
            if jt == 0 and b == 1 and g + 1 < c.NG:
                fetch_bias(g + 1)
            emit_s(g, b, jt)
            s_ptr += 1
        else:
            g, b, jt = iters[pv_ptr]
            emit_pv(g, b, jt)
            pv_ptr += 1
            if jt == c.NJT - 1:
                normalize(g, b)

    # ---- output projection (head-pair stacked, K=128) ---------------------
    # Emitted inside the attention scope, with PSUM borrowed from the ps_s
    # tags: those banks free right after the last exp, so the PE rolls from
    # the final PVs straight into the out matmuls without waiting for the
    # last block's accumulator evacuation (keeps the HAM clock-gate warm).
    ctx_out = ExitStack()
    outsb_pool = ctx_out.enter_context(tc.tile_pool(name="outsb", bufs=2))
    for b in range(c.BPC):
        for tt in range(c.NJT):
            tagn = "ps_sA" if (b * c.NJT + tt) % 2 == 0 else "ps_sB"
            nm = "ps_sA" if tagn == "ps_sA" else "ps_sB"
            ps_o2 = ps_s_pool.tile([128, c.V], FP32, name=nm, tag=tagn)
            ps_o = ps_o2[:, 0:c.D]
            nc.tensor.matmul(ps_o, ones_col[:], bo_bf[:],
                             start=True, stop=False)
            for g in range(c.NG):
                nc.tensor.matmul(
                    ps_o,
                    att_n[(g, b)][:, tt * 128:(tt + 1) * 128],
                    WoP[g][:],
                    start=False, stop=(g == c.NG - 1),
                )
            o_sb = outsb_pool.tile([128, c.D], FP32, name="o_sb", tag="o_sb")
            nc.scalar.activation(o_sb[:], ps_o,
                                 mybir.ActivationFunctionType.Copy)
            nc.sync.dma_start(out_d[b, tt * 128:(tt + 1) * 128, :], o_sb[:])

    ctx_out.close()
    ctx_att.close()
    ctx.close()


# --------------------------------------------------------------------------
# Host side
# --------------------------------------------------------------------------

def shard_inputs(x, Wq, Wk, Wv, Wo, bo, rpe, hop_matrix, cfg):
    c = cfg
    WqT = np.ascontiguousarray(Wq.T).astype(BF16_NP)
    WkT = np.ascontiguousarray(Wk.T).astype(BF16_NP)
    WvT = np.ascontiguousarray(Wv.T).astype(BF16_NP)
    WoT = np.ascontiguousarray(Wo.T).astype(BF16_NP)
    hopT = np.ascontiguousarray(hop_matrix.T).astype(BF16_NP)
    bo2 = np.ascontiguousarray(bo.astype(np.float32).reshape(1, c.D))
    in_maps = []
    for core in range(c.NC):
        xs = x[core * c.BPC:(core + 1) * c.BPC].astype(np.float32)
        xT = np.ascontiguousarray(xs.reshape(c.T, c.D).T).astype(BF16_NP)
        rpe_c = rpe[core].astype(np.float64)
        if GATHER_FP8:
            # softmax is invariant to scaling a head's exp-bias by alpha;
            # pick alpha to align exp(rpe) with the fp8(e4m3) grid
            e = np.exp(rpe_c)
            cands = np.linspace(1.0, 2.0, 513)[:-1]
            best_a, best_err = 1.0, np.inf
            for a in cands:
                q = np.asarray(a * e, dtype=ml_dtypes.float8_e4m3)
                err = np.sqrt(np.mean((q.astype(np.float64) / (a * e) - 1.0) ** 2))
                if err < best_err:
                    best_a, best_err = a, err
            rpe_c = rpe_c + np.log(best_a)
        in_maps.append({
            "xT": xT, "WqT": WqT, "WkT": WkT, "WvT": WvT, "WoT": WoT,
            "bo": bo2, "rpeh": np.ascontiguousarray(
                rpe_c.reshape(1, -1).astype(np.float32)),
            "hopT": hopT,
        })
    return in_maps


_CACHE = {}


def _get_compiled(cfg):
    key = (cfg.NC, cfg.B, cfg.V, cfg.D, cfg.H, cfg.NHOP)
    if key in _CACHE:
        return _CACHE[key]
    c = cfg
    nc = bacc.Bacc("TRN2", target_bir_lowering=False, debug=False,
                   num_devices=c.NC)
    ins = {
        "xT": nc.dram_tensor("xT", [c.D, c.T], BF16, kind="ExternalInput").ap(),
        "WqT": nc.dram_tensor("WqT", [c.D, c.D], BF16, kind="ExternalInput").ap(),
        "WkT": nc.dram_tensor("WkT", [c.D, c.D], BF16, kind="ExternalInput").ap(),
        "WvT": nc.dram_tensor("WvT", [c.D, c.D], BF16, kind="ExternalInput").ap(),
        "WoT": nc.dram_tensor("WoT", [c.D, c.D], BF16, kind="ExternalInput").ap(),
        "bo": nc.dram_tensor("bo", [1, c.D], FP32, kind="ExternalInput").ap(),
        "rpeh": nc.dram_tensor("rpeh", [1, c.NHOP], FP32,
                               kind="ExternalInput").ap(),
        "hopT": nc.dram_tensor("hopT", [c.V, c.V], BF16,
                               kind="ExternalInput").ap(),
    }
    out = nc.dram_tensor("out", [c.BPC, c.V, c.D], FP32,
                         kind="ExternalOutput").ap()
    with tile.TileContext(nc) as tc:
        build_graph(tc, out, ins, cfg)
    nc.compile()
    _CACHE[key] = nc
    return nc


def kernel(x, Wq, Wk, Wv, Wo, bo, rpe, hop_matrix):
    from concourse.bass_utils import run_bass_kernel_spmd

    cfg = Cfg(N_CORES, B, V, D, H, NHOP)
    nc = _get_compiled(cfg)
    in_maps = shard_inputs(np.asarray(x), np.asarray(Wq), np.asarray(Wk),
                           np.asarray(Wv), np.asarray(Wo), np.asarray(bo),
                           np.asarray(rpe), np.asarray(hop_matrix), cfg)
    res = run_bass_kernel_spmd(nc, in_maps, core_ids=list(range(cfg.NC)))
    return np.concatenate([res.results[c]["out"] for c in range(cfg.NC)],
                          axis=0)
